# revision 13
# baseline (speedup 1.0000x reference)
"""Trainium2 Bass kernel for nn_AttributeDecoder (gather + per-head small linear).

  logits[k, s, v] = features.reshape(-1, 256)[mask_idx[k, s], :] @ W[k] + b[k]
  K=24 heads, S=16384 positions/head, D=256, V=8, N=131072 table rows.

Sharding: expert-parallel over heads — 3 heads per core x 8 cores, features
table replicated (per-core DRAM copy, gathered via dma_gather).

Per (core, head): indices bucketed by table region (so in-bucket offsets fit
dma_gather's int16 indices), padded to a fixed bucket size.

bf16t mode (default): features stored bf16 (512B rows — half the gather
traffic of fp32, and exactly at the DMA engines' full-rate transfer size).
dma_gather(transpose=True) delivers rows pre-transposed (D on partitions),
so each <=512-position block is just 2 bf16 matmuls (the two 128-row D
chunks) + 1 bias-row matmul (ones vector) accumulated in PSUM, then a
PSUM->SBUF copy alternating Vector/Scalar engines and one batched DMA out
per bucket. absmax err ~1e-3 relative, well under the 2e-2 gate.

fp32 mode (fallback, exact): non-transposed fp32 gather -> PE transpose ->
fp32 matmul.

Host unpermutes the bucketed output order.
"""
import os
import numpy as np

import concourse.bass as bass
import concourse.mybir as mybir
import concourse.tile as tile
from concourse import bacc
from concourse.bass_utils import run_bass_kernel_spmd
from concourse.masks import make_identity

NCORES = 8
KH = 3                 # heads per core
P = 128
D = 256
V = 8
S = 16384
NROWS = 131072

MODE = os.environ.get("KERNEL_MODE", "bf16t")   # "bf16t" | "fp32" (exact)

# per-mode gather geometry
GEO = {
    # NBUCK buckets of NROWS/NBUCK rows; BS padded positions per bucket;
    # BLOCKS: matmul n-block widths covering BS positions
    "fp32":  dict(NBUCK=4, BS=4608, BLOCKS=[512] * 9),
    "bf16t": dict(NBUCK=4, BS=4352, BLOCKS=[512] * 8 + [256]),
}
# sub-gather sizes for bf16t: multiples of 512 so matmul blocks never span
# chunk tiles (each sub-gather writes its own contiguous tile). All gathers
# MUST share one SWDGE queue: concurrent queues complete out of order and
# break the tile framework's cumulative DMA-completion semaphores (observed
# as block-granular garbage on HW).
QSPLIT = [4352]
assert sum(QSPLIT) == GEO["bf16t"]["BS"]
for g in GEO.values():
    g["SW"] = g["NBUCK"] * g["BS"]
    assert sum(g["BLOCKS"]) == g["BS"]

f32 = mybir.dt.float32
bf16 = mybir.dt.bfloat16
i16 = mybir.dt.int16

_NC_CACHE = {}


def build_nc(mode=MODE, loop_k=None):
    NBUCK = GEO[mode]["NBUCK"]
    BS = GEO[mode]["BS"]
    BLOCKS = GEO[mode]["BLOCKS"]
    SW = GEO[mode]["SW"]
    BROWS = NROWS // NBUCK
    HB = KH * NBUCK
    TILES = BS // P

    nc = bacc.Bacc("TRN2", target_bir_lowering=False, debug=False,
                   num_swdge_queues=4)
    if mode == "fp32":
        feat = nc.dram_tensor("feat", [NROWS, D], f32, kind="ExternalInput")
        w = nc.dram_tensor("w", [P, KH * 2 * V], f32, kind="ExternalInput")
    else:
        feat = nc.dram_tensor("feat", [NROWS, D], bf16, kind="ExternalInput")
        w = nc.dram_tensor("w", [P, KH * 2 * V], bf16, kind="ExternalInput")
        # bias folded into a PE matmul: row 0 of biasw carries the bias,
        # rows 1..127 are zero, against an all-ones rhs — full 128-partition
        # contraction (1-partition matmuls misbehave on HW)
        biasw = nc.dram_tensor("biasw", [P, KH * V], bf16, kind="ExternalInput")
        ones = nc.dram_tensor("ones", [P, 512], bf16, kind="ExternalInput")
    idx = nc.dram_tensor("idx", [HB, P, BS // 16], i16, kind="ExternalInput")
    if mode == "fp32":
        bias = nc.dram_tensor("bias", [V, KH], f32, kind="ExternalInput")
    out = nc.dram_tensor("out", [KH, V, SW], f32, kind="ExternalOutput")

    with tile.TileContext(nc) as tc:
        with tc.tile_pool(name="const", bufs=1) as cpool, \
             tc.tile_pool(name="gath", bufs=3) as gpool, \
             tc.tile_pool(name="gt", bufs=4) as gtpool, \
             tc.tile_pool(name="ob", bufs=2) as obpool, \
             tc.tile_pool(name="pst", bufs=3, space="PSUM") as ptpool, \
             tc.tile_pool(name="pso", bufs=4, space="PSUM") as popool:

            w_sb = cpool.tile(list(w.shape), w.dtype)
            nc.sync.dma_start(w_sb[:], w[:])
            idx_sb = cpool.tile([P, HB, BS // 16], i16)
            for hb in range(HB):
                nc.sync.dma_start(idx_sb[:, hb, :], idx[hb])
            ident = None
            if mode == "fp32":
                bias_sb = cpool.tile([V, KH], f32)
                nc.sync.dma_start(bias_sb[:], bias[:])
                ident = cpool.tile([P, P], f32)
                make_identity(nc, ident[:])
            else:
                biasw_sb = cpool.tile([P, KH * V], bf16)
                nc.sync.dma_start(biasw_sb[:], biasw[:])
                ones_sb = cpool.tile([P, 512], bf16)
                nc.sync.dma_start(ones_sb[:], ones[:])

            import contextlib
            loop_cm = tc.For_i(0, loop_k, 1) if loop_k else contextlib.nullcontext()
            with loop_cm:
                for h in range(KH):
                    for b in range(NBUCK):
                        hb = h * NBUCK + b
                        if mode == "fp32":
                            g = gpool.tile([P, TILES, D], f32, tag="g")
                            # split across all 4 SWDGE queues for more
                            # outstanding HBM reads
                            HT = TILES // 4
                            HN = BS // 4
                            for q in range(4):
                                nc.gpsimd.dma_gather(
                                    g[:, q * HT:(q + 1) * HT, :],
                                    feat[b * BROWS:(b + 1) * BROWS, :],
                                    idx_sb[:, hb, q * (HN // 16):(q + 1) * (HN // 16)],
                                    HN, HN, D,
                                    single_packet=False, queue_num=q)
                        else:
                            # split across all 4 SWDGE queues, same queue set
                            # every bucket: queue-FIFO keeps cross-bucket
                            # completion ordered (one gather per bucket on
                            # rotating queues raced on HW). One tile per
                            # queue chunk (transpose gather needs contiguous
                            # free dims), so matmuls start per-chunk.
                            gs = []
                            off_q = 0
                            for q, NQ in enumerate(QSPLIT):
                                gq = gpool.tile([P, 2, NQ], bf16, tag=f"g{q}")
                                nc.gpsimd.dma_gather(
                                    gq[:], feat[b * BROWS:(b + 1) * BROWS, :],
                                    idx_sb[:, hb, off_q // 16:(off_q + NQ) // 16],
                                    NQ, NQ, D,
                                    transpose=True, single_packet=False,
                                    queue_num=0)
                                gs.append((gq, off_q, NQ))
                                off_q += NQ
                            ob = obpool.tile([V, BS], f32, tag="ob")
                            blk = 0
                            for gq, qoff, NQ in gs:
                                loff = 0
                                while loff < NQ:
                                    NW = min(512, NQ - loff)
                                    sl = slice(loff, loff + NW)
                                    po = popool.tile([V, 512], f32, tag="po")
                                    nc.tensor.matmul(
                                        po[:, :NW],
                                        lhsT=w_sb[:, (h * 2 + 0) * V:(h * 2 + 1) * V],
                                        rhs=gq[:, 0, sl], start=True, stop=False)
                                    nc.tensor.matmul(
                                        po[:, :NW],
                                        lhsT=w_sb[:, (h * 2 + 1) * V:(h * 2 + 2) * V],
                                        rhs=gq[:, 1, sl], start=False, stop=False)
                                    nc.tensor.matmul(
                                        po[:, :NW],
                                        lhsT=biasw_sb[:, h * V:(h + 1) * V],
                                        rhs=ones_sb[:, :NW],
                                        start=False, stop=True)
                                    osl = slice(qoff + loff, qoff + loff + NW)
                                    # alternate PSUM->SBUF copy between the
                                    # two free engines
                                    if blk % 2 == 0:
                                        nc.vector.tensor_copy(ob[:, osl], po[:, :NW])
                                    else:
                                        nc.scalar.activation(
                                            ob[:, osl], po[:V, :NW],
                                            mybir.ActivationFunctionType.Identity)
                                    blk += 1
                                    loff += NW
                            nc.sync.dma_start(out[h, :, b * BS:(b + 1) * BS],
                                              ob[:])
                            continue
                        off = 0
                        for blk, NW in enumerate(BLOCKS):
                            if mode == "fp32":
                                pt0 = ptpool.tile([P, 512], f32, tag="pt0")
                                pt1 = ptpool.tile([P, 512], f32, tag="pt1")
                                for tl in range(NW // P):
                                    t = off // P + tl
                                    nc.tensor.transpose(
                                        out=pt0[:, tl * P:(tl + 1) * P],
                                        in_=g[:, t, 0:P], identity=ident[:])
                                    nc.tensor.transpose(
                                        out=pt1[:, tl * P:(tl + 1) * P],
                                        in_=g[:, t, P:D], identity=ident[:])
                                gt0 = gtpool.tile([P, 512], f32, tag="gt0")
                                gt1 = gtpool.tile([P, 512], f32, tag="gt1")
                                nc.vector.tensor_copy(gt0[:, :NW], pt0[:, :NW])
                                nc.vector.tensor_copy(gt1[:, :NW], pt1[:, :NW])
                                po = popool.tile([V, 512], f32, tag="po")
                                nc.tensor.matmul(
                                    po[:, :NW],
                                    lhsT=w_sb[:, (h * 2 + 0) * V:(h * 2 + 1) * V],
                                    rhs=gt0[:, :NW], start=True, stop=False)
                                nc.tensor.matmul(
                                    po[:, :NW],
                                    lhsT=w_sb[:, (h * 2 + 1) * V:(h * 2 + 2) * V],
                                    rhs=gt1[:, :NW], start=False, stop=True)
                                ob = obpool.tile([V, 512], f32, tag="ob")
                                nc.scalar.activation(
                                    ob[:, :NW], po[:V, :NW],
                                    mybir.ActivationFunctionType.Identity,
                                    bias=bias_sb[:, h:h + 1])
                                nc.sync.dma_start(
                                    out[h, :, b * BS + off: b * BS + off + NW],
                                    ob[:, :NW])
                            off += NW
    nc.compile()
    return nc


def get_nc(mode=MODE):
    if mode not in _NC_CACHE:
        _NC_CACHE[mode] = build_nc(mode)
    return _NC_CACHE[mode]


def _wrap_idx(a, BS):
    """[BS] int16 -> [P, BS//16]: idx i at [i % 16, i // 16], replicated x8."""
    return np.tile(a.reshape(BS // 16, 16).T, (8, 1))


def prep_inputs(features, mask_idx, head_weights, head_bias, mode=MODE):
    """Build per-core in_maps + the unpermute info."""
    NBUCK = GEO[mode]["NBUCK"]
    BS = GEO[mode]["BS"]
    HB = KH * NBUCK
    shift = {4: 15, 8: 14}[NBUCK]
    mask = (1 << shift) - 1

    feats = np.ascontiguousarray(
        np.asarray(features, dtype=np.float32).reshape(NROWS, D))
    mask_idx = np.asarray(mask_idx, dtype=np.int32)
    W = np.asarray(head_weights, dtype=np.float32)
    hbias = np.asarray(head_bias, dtype=np.float32)

    if mode == "fp32":
        feat_in = feats
    else:
        import ml_dtypes
        feat_in = feats.astype(ml_dtypes.bfloat16)

    in_maps = []
    unperm = []   # per head: (order, counts)
    for c in range(NCORES):
        idx_payload = np.zeros((HB, P, BS // 16), np.int16)
        for hidx, k in enumerate(range(c * KH, (c + 1) * KH)):
            gid = mask_idx[k]
            bidx = gid >> shift
            # sort by full index (not just bucket id): within-bucket ascending
            # order gives the SDMA engines HBM-page-local read streams
            order = np.argsort(gid, kind="stable")
            counts = np.bincount(bidx, minlength=NBUCK)
            assert counts.max() <= BS, f"bucket overflow: {counts}"
            rel = (gid & mask).astype(np.int16)
            pos = 0
            for bb in range(NBUCK):
                cnt = int(counts[bb])
                padded = np.zeros(BS, np.int16)
                padded[:cnt] = rel[order[pos:pos + cnt]]
                idx_payload[hidx * NBUCK + bb] = _wrap_idx(padded, BS)
                pos += cnt
            unperm.append((order, counts))

        Wc = W[c * KH:(c + 1) * KH]          # [KH, 256, 8]
        w_in = np.ascontiguousarray(
            Wc.reshape(KH, 2, P, V).transpose(2, 0, 1, 3).reshape(P, KH * 2 * V))
        if mode == "fp32":
            bias_in = np.ascontiguousarray(hbias[c * KH:(c + 1) * KH].T)  # [V, KH]
            in_maps.append({"feat": feat_in, "idx": idx_payload,
                            "w": w_in, "bias": bias_in})
        else:
            import ml_dtypes
            biasw_in = np.zeros((P, KH * V), np.float32)
            biasw_in[0] = hbias[c * KH:(c + 1) * KH].reshape(KH * V)
            in_maps.append({
                "feat": feat_in, "idx": idx_payload,
                "w": w_in.astype(ml_dtypes.bfloat16),
                "biasw": biasw_in.astype(ml_dtypes.bfloat16),
                "ones": np.ones((P, 512), ml_dtypes.bfloat16),
            })
    return in_maps, unperm


def assemble_output(results, unperm, mode=MODE):
    NBUCK = GEO[mode]["NBUCK"]
    BS = GEO[mode]["BS"]
    out_full = np.empty((NCORES * KH, S, V), np.float32)
    for c in range(NCORES):
        dev = results[c]["out"]              # [KH, V, SW]
        for h in range(KH):
            k = c * KH + h
            order, counts = unperm[k]
            cols = np.concatenate(
                [np.arange(bb * BS, bb * BS + counts[bb]) for bb in range(NBUCK)])
            out_full[k, order, :] = dev[h][:, cols].T
    return out_full


def kernel(block_type_grid=None, features=None, mask_idx=None,
           head_weights=None, head_bias=None):
    nc = get_nc(MODE)
    in_maps, unperm = prep_inputs(features, mask_idx, head_weights, head_bias, MODE)
    res = run_bass_kernel_spmd(nc, in_maps, list(range(NCORES)))
    return assemble_output(res.results, unperm, MODE)


# revision 19
# speedup vs baseline: 2.9116x; 2.9116x over previous
"""Trainium2 Bass kernel for nn_AttributeDecoder (gather + per-head small linear).

  logits[k, s, v] = features.reshape(-1, 256)[mask_idx[k, s], :] @ W[k] + b[k]
  K=24 heads, S=16384 positions/head, D=256, V=8, N=131072 table rows.

Sharding: expert-parallel over heads — 3 heads per core x 8 cores, features
table replicated (per-core DRAM copy, gathered via dma_gather).

Per (core, head): indices bucketed by table region (so in-bucket offsets fit
dma_gather's int16 indices), padded to a fixed bucket size.

bf16t mode (default): features stored bf16 (512B rows — half the gather
traffic of fp32, and exactly at the DMA engines' full-rate transfer size).
dma_gather(transpose=True) delivers rows pre-transposed (D on partitions),
so each <=512-position block is just 2 bf16 matmuls (the two 128-row D
chunks) + 1 bias-row matmul (ones vector) accumulated in PSUM, then a
PSUM->SBUF copy alternating Vector/Scalar engines and one batched DMA out
per bucket. absmax err ~1e-3 relative, well under the 2e-2 gate.

fp32 mode (fallback, exact): non-transposed fp32 gather -> PE transpose ->
fp32 matmul.

Host unpermutes the bucketed output order.
"""
import os
import numpy as np

import concourse.bass as bass
import concourse.mybir as mybir
import concourse.tile as tile
from concourse import bacc
from concourse.bass_utils import run_bass_kernel_spmd
from concourse.masks import make_identity

NCORES = 8
KH = 3                 # heads per core
P = 128
D = 256
V = 8
S = 16384
NROWS = 131072

MODE = os.environ.get("KERNEL_MODE", "bf16n")   # "bf16n" | "bf16t" | "fp32"

# per-mode gather geometry
GEO = {
    # NBUCK buckets of NROWS/NBUCK rows; BS padded positions per bucket;
    # BLOCKS: matmul n-block widths covering BS positions
    "fp32":  dict(NBUCK=4, BS=4608, BLOCKS=[512] * 9),
    "bf16t": dict(NBUCK=4, BS=4352, BLOCKS=[512] * 8 + [256]),
    "bf16n": dict(NBUCK=4, BS=4352, BLOCKS=[512] * 8 + [256]),
}
# sub-gather sizes for bf16t: multiples of 512 so matmul blocks never span
# chunk tiles (each sub-gather writes its own contiguous tile). All gathers
# MUST share one SWDGE queue: concurrent queues complete out of order and
# break the tile framework's cumulative DMA-completion semaphores (observed
# as block-granular garbage on HW).
QSPLIT = [4352]
assert sum(QSPLIT) == GEO["bf16t"]["BS"]
# bf16n non-transpose sub-gathers: 4 queues (multi-queue is safe for
# NON-transpose gathers — the fp32 baseline proved it on HW — and is 3x
# faster than one queue since each queue feeds its own DMA rings).
# Sizes are multiples of 128 so each sub-gather covers whole 128-row tiles.
NSPLIT = [1152, 1024, 1152, 1024]
assert sum(NSPLIT) == GEO["bf16n"]["BS"]
for g in GEO.values():
    g["SW"] = g["NBUCK"] * g["BS"]
    assert sum(g["BLOCKS"]) == g["BS"]

f32 = mybir.dt.float32
bf16 = mybir.dt.bfloat16
i16 = mybir.dt.int16

_NC_CACHE = {}


def build_nc(mode=MODE, loop_k=None):
    NBUCK = GEO[mode]["NBUCK"]
    BS = GEO[mode]["BS"]
    BLOCKS = GEO[mode]["BLOCKS"]
    SW = GEO[mode]["SW"]
    BROWS = NROWS // NBUCK
    HB = KH * NBUCK
    TILES = BS // P

    nc = bacc.Bacc("TRN2", target_bir_lowering=False, debug=False,
                   num_swdge_queues=4)
    if mode == "fp32":
        feat = nc.dram_tensor("feat", [NROWS, D], f32, kind="ExternalInput")
        w = nc.dram_tensor("w", [P, KH * 2 * V], f32, kind="ExternalInput")
    else:
        feat = nc.dram_tensor("feat", [NROWS, D], bf16, kind="ExternalInput")
        w = nc.dram_tensor("w", [P, KH * 2 * V], bf16, kind="ExternalInput")
        # bias folded into a PE matmul: row 0 of biasw carries the bias,
        # rows 1..127 are zero, against an all-ones rhs — full 128-partition
        # contraction (1-partition matmuls misbehave on HW)
        biasw = nc.dram_tensor("biasw", [P, KH * V], bf16, kind="ExternalInput")
        ones = nc.dram_tensor("ones", [P, 512], bf16, kind="ExternalInput")

    idx = nc.dram_tensor("idx", [HB, P, BS // 16], i16, kind="ExternalInput")
    if mode == "fp32":
        bias = nc.dram_tensor("bias", [V, KH], f32, kind="ExternalInput")
    out = nc.dram_tensor("out", [KH, V, SW], f32, kind="ExternalOutput")

    with tile.TileContext(nc) as tc:
        with tc.tile_pool(name="const", bufs=1) as cpool, \
             tc.tile_pool(name="gath", bufs=3) as gpool, \
             tc.tile_pool(name="gt", bufs=4) as gtpool, \
             tc.tile_pool(name="ob", bufs=2) as obpool, \
             tc.tile_pool(name="pst", bufs=3, space="PSUM") as ptpool, \
             tc.tile_pool(name="pso", bufs=4, space="PSUM") as popool:

            w_sb = cpool.tile(list(w.shape), w.dtype)
            nc.sync.dma_start(w_sb[:], w[:])
            idx_sb = cpool.tile([P, HB, BS // 16], i16)
            for hb in range(HB):
                nc.sync.dma_start(idx_sb[:, hb, :], idx[hb])
            ident = None
            if mode == "fp32":
                bias_sb = cpool.tile([V, KH], f32)
                nc.sync.dma_start(bias_sb[:], bias[:])
                ident = cpool.tile([P, P], f32)
                make_identity(nc, ident[:])
            else:
                biasw_sb = cpool.tile([P, KH * V], bf16)
                nc.sync.dma_start(biasw_sb[:], biasw[:])
                ones_sb = cpool.tile([P, 512], bf16)
                nc.sync.dma_start(ones_sb[:], ones[:])
                if mode == "bf16n":
                    ident = cpool.tile([P, P], bf16)
                    make_identity(nc, ident[:])

            import contextlib
            loop_cm = tc.For_i(0, loop_k, 1) if loop_k else contextlib.nullcontext()
            with loop_cm:
                for h in range(KH):
                    for b in range(NBUCK):
                        hb = h * NBUCK + b
                        if mode == "fp32":
                            g = gpool.tile([P, TILES, D], f32, tag="g")
                            # split across all 4 SWDGE queues for more
                            # outstanding HBM reads
                            HT = TILES // 4
                            HN = BS // 4
                            for q in range(4):
                                nc.gpsimd.dma_gather(
                                    g[:, q * HT:(q + 1) * HT, :],
                                    feat[b * BROWS:(b + 1) * BROWS, :],
                                    idx_sb[:, hb, q * (HN // 16):(q + 1) * (HN // 16)],
                                    HN, HN, D,
                                    single_packet=False, queue_num=q)
                        elif mode == "bf16n":
                            # non-transpose bf16 gather, 4 queues (multi-queue
                            # is HW-safe for non-transpose gathers), then PE
                            # transposes each [128,128] tile (1 cyc/row bf16)
                            g = gpool.tile([P, TILES, D], bf16, tag="g")
                            toff = 0
                            for q, NQ in enumerate(NSPLIT):
                                nc.gpsimd.dma_gather(
                                    g[:, toff // P:(toff + NQ) // P, :],
                                    feat[b * BROWS:(b + 1) * BROWS, :],
                                    idx_sb[:, hb, toff // 16:(toff + NQ) // 16],
                                    NQ, NQ, D,
                                    single_packet=False, queue_num=q)
                                toff += NQ
                            ob = obpool.tile([V, BS], f32, tag="ob")
                            off = 0
                            for blk, NW in enumerate(BLOCKS):
                                # both 128-row D chunks transposed into one
                                # PSUM bank: [:, 0:512] = D 0:128,
                                # [:, 512:1024] = D 128:256
                                pt = ptpool.tile([P, 1024], bf16, tag="pt")
                                for tl in range(NW // P):
                                    t = off // P + tl
                                    nc.tensor.transpose(
                                        out=pt[:, tl * P:(tl + 1) * P],
                                        in_=g[:, t, 0:P], identity=ident[:])
                                    nc.tensor.transpose(
                                        out=pt[:, 512 + tl * P:512 + (tl + 1) * P],
                                        in_=g[:, t, P:D], identity=ident[:])
                                gt = gtpool.tile([P, 1024], bf16, tag="gt")
                                nc.vector.tensor_copy(gt[:, :NW], pt[:, :NW])
                                nc.scalar.activation(
                                    gt[:, 512:512 + NW], pt[:, 512:512 + NW],
                                    mybir.ActivationFunctionType.Identity)
                                po = popool.tile([V, 512], f32, tag="po")
                                nc.tensor.matmul(
                                    po[:, :NW],
                                    lhsT=w_sb[:, (h * 2 + 0) * V:(h * 2 + 1) * V],
                                    rhs=gt[:, :NW], start=True, stop=False)
                                nc.tensor.matmul(
                                    po[:, :NW],
                                    lhsT=w_sb[:, (h * 2 + 1) * V:(h * 2 + 2) * V],
                                    rhs=gt[:, 512:512 + NW], start=False, stop=False)
                                nc.tensor.matmul(
                                    po[:, :NW],
                                    lhsT=biasw_sb[:, h * V:(h + 1) * V],
                                    rhs=ones_sb[:, :NW],
                                    start=False, stop=True)
                                if blk % 2 == 0:
                                    nc.vector.tensor_copy(
                                        ob[:, off:off + NW], po[:, :NW])
                                else:
                                    nc.scalar.activation(
                                        ob[:, off:off + NW], po[:V, :NW],
                                        mybir.ActivationFunctionType.Identity)
                                off += NW
                            nc.sync.dma_start(out[h, :, b * BS:(b + 1) * BS],
                                              ob[:])
                            continue
                        else:
                            # split across all 4 SWDGE queues, same queue set
                            # every bucket: queue-FIFO keeps cross-bucket
                            # completion ordered (one gather per bucket on
                            # rotating queues raced on HW). One tile per
                            # queue chunk (transpose gather needs contiguous
                            # free dims), so matmuls start per-chunk.
                            gs = []
                            off_q = 0
                            for q, NQ in enumerate(QSPLIT):
                                gq = gpool.tile([P, 2, NQ], bf16, tag=f"g{q}")
                                nc.gpsimd.dma_gather(
                                    gq[:], feat[b * BROWS:(b + 1) * BROWS, :],
                                    idx_sb[:, hb, off_q // 16:(off_q + NQ) // 16],
                                    NQ, NQ, D,
                                    transpose=True, single_packet=False,
                                    queue_num=0)
                                gs.append((gq, off_q, NQ))
                                off_q += NQ
                            ob = obpool.tile([V, BS], f32, tag="ob")
                            blk = 0
                            for gq, qoff, NQ in gs:
                                loff = 0
                                while loff < NQ:
                                    NW = min(512, NQ - loff)
                                    sl = slice(loff, loff + NW)
                                    po = popool.tile([V, 512], f32, tag="po")
                                    nc.tensor.matmul(
                                        po[:, :NW],
                                        lhsT=w_sb[:, (h * 2 + 0) * V:(h * 2 + 1) * V],
                                        rhs=gq[:, 0, sl], start=True, stop=False)
                                    nc.tensor.matmul(
                                        po[:, :NW],
                                        lhsT=w_sb[:, (h * 2 + 1) * V:(h * 2 + 2) * V],
                                        rhs=gq[:, 1, sl], start=False, stop=False)
                                    nc.tensor.matmul(
                                        po[:, :NW],
                                        lhsT=biasw_sb[:, h * V:(h + 1) * V],
                                        rhs=ones_sb[:, :NW],
                                        start=False, stop=True)
                                    osl = slice(qoff + loff, qoff + loff + NW)
                                    # alternate PSUM->SBUF copy between the
                                    # two free engines
                                    if blk % 2 == 0:
                                        nc.vector.tensor_copy(ob[:, osl], po[:, :NW])
                                    else:
                                        nc.scalar.activation(
                                            ob[:, osl], po[:V, :NW],
                                            mybir.ActivationFunctionType.Identity)
                                    blk += 1
                                    loff += NW
                            nc.sync.dma_start(out[h, :, b * BS:(b + 1) * BS],
                                              ob[:])
                            continue
                        off = 0
                        for blk, NW in enumerate(BLOCKS):
                            if mode == "fp32":
                                pt0 = ptpool.tile([P, 512], f32, tag="pt0")
                                pt1 = ptpool.tile([P, 512], f32, tag="pt1")
                                for tl in range(NW // P):
                                    t = off // P + tl
                                    nc.tensor.transpose(
                                        out=pt0[:, tl * P:(tl + 1) * P],
                                        in_=g[:, t, 0:P], identity=ident[:])
                                    nc.tensor.transpose(
                                        out=pt1[:, tl * P:(tl + 1) * P],
                                        in_=g[:, t, P:D], identity=ident[:])
                                gt0 = gtpool.tile([P, 512], f32, tag="gt0")
                                gt1 = gtpool.tile([P, 512], f32, tag="gt1")
                                nc.vector.tensor_copy(gt0[:, :NW], pt0[:, :NW])
                                nc.vector.tensor_copy(gt1[:, :NW], pt1[:, :NW])
                                po = popool.tile([V, 512], f32, tag="po")
                                nc.tensor.matmul(
                                    po[:, :NW],
                                    lhsT=w_sb[:, (h * 2 + 0) * V:(h * 2 + 1) * V],
                                    rhs=gt0[:, :NW], start=True, stop=False)
                                nc.tensor.matmul(
                                    po[:, :NW],
                                    lhsT=w_sb[:, (h * 2 + 1) * V:(h * 2 + 2) * V],
                                    rhs=gt1[:, :NW], start=False, stop=True)
                                ob = obpool.tile([V, 512], f32, tag="ob")
                                nc.scalar.activation(
                                    ob[:, :NW], po[:V, :NW],
                                    mybir.ActivationFunctionType.Identity,
                                    bias=bias_sb[:, h:h + 1])
                                nc.sync.dma_start(
                                    out[h, :, b * BS + off: b * BS + off + NW],
                                    ob[:, :NW])
                            off += NW
    nc.compile()
    return nc


def get_nc(mode=MODE):
    if mode not in _NC_CACHE:
        _NC_CACHE[mode] = build_nc(mode)
    return _NC_CACHE[mode]


def _wrap_idx(a, BS):
    """[BS] int16 -> [P, BS//16]: idx i at [i % 16, i // 16], replicated x8."""
    return np.tile(a.reshape(BS // 16, 16).T, (8, 1))


def prep_inputs(features, mask_idx, head_weights, head_bias, mode=MODE):
    """Build per-core in_maps + the unpermute info."""
    NBUCK = GEO[mode]["NBUCK"]
    BS = GEO[mode]["BS"]
    HB = KH * NBUCK
    shift = {4: 15, 8: 14}[NBUCK]
    mask = (1 << shift) - 1

    feats = np.ascontiguousarray(
        np.asarray(features, dtype=np.float32).reshape(NROWS, D))
    mask_idx = np.asarray(mask_idx, dtype=np.int32)
    W = np.asarray(head_weights, dtype=np.float32)
    hbias = np.asarray(head_bias, dtype=np.float32)

    if mode == "fp32":
        feat_in = feats
    else:
        import ml_dtypes
        feat_in = feats.astype(ml_dtypes.bfloat16)

    in_maps = []
    unperm = []   # per head: (order, counts)
    for c in range(NCORES):
        idx_payload = np.zeros((HB, P, BS // 16), np.int16)
        for hidx, k in enumerate(range(c * KH, (c + 1) * KH)):
            gid = mask_idx[k]
            bidx = gid >> shift
            # sort by full index (not just bucket id): within-bucket ascending
            # order gives the SDMA engines HBM-page-local read streams
            order = np.argsort(gid, kind="stable")
            counts = np.bincount(bidx, minlength=NBUCK)
            assert counts.max() <= BS, f"bucket overflow: {counts}"
            rel = (gid & mask).astype(np.int16)
            pos = 0
            for bb in range(NBUCK):
                cnt = int(counts[bb])
                padded = np.zeros(BS, np.int16)
                padded[:cnt] = rel[order[pos:pos + cnt]]
                idx_payload[hidx * NBUCK + bb] = _wrap_idx(padded, BS)
                pos += cnt
            unperm.append((order, counts))

        Wc = W[c * KH:(c + 1) * KH]          # [KH, 256, 8]
        w_in = np.ascontiguousarray(
            Wc.reshape(KH, 2, P, V).transpose(2, 0, 1, 3).reshape(P, KH * 2 * V))
        if mode == "fp32":
            bias_in = np.ascontiguousarray(hbias[c * KH:(c + 1) * KH].T)  # [V, KH]
            in_maps.append({"feat": feat_in, "idx": idx_payload,
                            "w": w_in, "bias": bias_in})
        else:
            import ml_dtypes
            biasw_in = np.zeros((P, KH * V), np.float32)
            biasw_in[0] = hbias[c * KH:(c + 1) * KH].reshape(KH * V)
            in_maps.append({
                "feat": feat_in, "idx": idx_payload,
                "w": w_in.astype(ml_dtypes.bfloat16),
                "biasw": biasw_in.astype(ml_dtypes.bfloat16),
                "ones": np.ones((P, 512), ml_dtypes.bfloat16),
            })
    return in_maps, unperm


def assemble_output(results, unperm, mode=MODE):
    NBUCK = GEO[mode]["NBUCK"]
    BS = GEO[mode]["BS"]
    out_full = np.empty((NCORES * KH, S, V), np.float32)
    for c in range(NCORES):
        dev = results[c]["out"]              # [KH, V, SW]
        for h in range(KH):
            k = c * KH + h
            order, counts = unperm[k]
            cols = np.concatenate(
                [np.arange(bb * BS, bb * BS + counts[bb]) for bb in range(NBUCK)])
            out_full[k, order, :] = dev[h][:, cols].T
    return out_full


def kernel(block_type_grid=None, features=None, mask_idx=None,
           head_weights=None, head_bias=None):
    nc = get_nc(MODE)
    in_maps, unperm = prep_inputs(features, mask_idx, head_weights, head_bias, MODE)
    res = run_bass_kernel_spmd(nc, in_maps, list(range(NCORES)))
    return assemble_output(res.results, unperm, MODE)


# revision 28
# speedup vs baseline: 3.8765x; 1.3314x over previous
"""Trainium2 Bass kernel for nn_AttributeDecoder (gather + per-head small linear).

  logits[k, s, v] = features.reshape(-1, 256)[mask_idx[k, s], :] @ W[k] + b[k]
  K=24 heads, S=16384 positions/head, D=256, V=8, N=131072 table rows.

Sharding: expert-parallel over heads — 3 heads per core x 8 cores, features
table replicated (per-core DRAM copy, gathered via dma_gather).

Per (core, head): indices bucketed by table region (so in-bucket offsets fit
dma_gather's int16 indices), padded to a fixed bucket size.

bf16t mode (default): features stored bf16 (512B rows — half the gather
traffic of fp32, and exactly at the DMA engines' full-rate transfer size).
dma_gather(transpose=True) delivers rows pre-transposed (D on partitions),
so each <=512-position block is just 2 bf16 matmuls (the two 128-row D
chunks) + 1 bias-row matmul (ones vector) accumulated in PSUM, then a
PSUM->SBUF copy alternating Vector/Scalar engines and one batched DMA out
per bucket. absmax err ~1e-3 relative, well under the 2e-2 gate.

fp32 mode (fallback, exact): non-transposed fp32 gather -> PE transpose ->
fp32 matmul.

Host unpermutes the bucketed output order.
"""
import os
import numpy as np

import concourse.bass as bass
import concourse.mybir as mybir
import concourse.tile as tile
from concourse import bacc
from concourse.bass_utils import run_bass_kernel_spmd
from concourse.masks import make_identity

NCORES = 8
KH = 3                 # heads per core
P = 128
D = 256
V = 8
S = 16384
NROWS = 131072

MODE = os.environ.get("KERNEL_MODE", "bf16u")   # bf16u|bf16n|bf16t|fp32

# per-mode gather geometry
GEO = {
    # NBUCK buckets of NROWS/NBUCK rows; BS padded positions per bucket;
    # BLOCKS: matmul n-block widths covering BS positions
    "fp32":  dict(NBUCK=4, BS=4608, BLOCKS=[512] * 9),
    "bf16t": dict(NBUCK=4, BS=4352, BLOCKS=[512] * 8 + [256]),
    "bf16n": dict(NBUCK=4, BS=4352, BLOCKS=[512] * 8 + [256]),
    # bf16u: per-bucket gather of the UNION of the core's 3 heads' rows
    # (~17% fewer rows), all 24 logits computed per gathered row
    "bf16u": dict(NBUCK=8, BS=5248, BLOCKS=[512] * 10 + [128]),
}
# sub-gather sizes for bf16t: multiples of 512 so matmul blocks never span
# chunk tiles (each sub-gather writes its own contiguous tile). All gathers
# MUST share one SWDGE queue: concurrent queues complete out of order and
# break the tile framework's cumulative DMA-completion semaphores (observed
# as block-granular garbage on HW).
QSPLIT = [4352]
assert sum(QSPLIT) == GEO["bf16t"]["BS"]
# bf16n non-transpose sub-gathers: 4 queues (multi-queue is safe for
# NON-transpose gathers — the fp32 baseline proved it on HW — and is 3x
# faster than one queue since each queue feeds its own DMA rings).
# Sizes are multiples of 128 so each sub-gather covers whole 128-row tiles.
NSPLIT = [1152, 1024, 1152, 1024]
assert sum(NSPLIT) == GEO["bf16n"]["BS"]
USPLIT = [1408, 1280, 1280, 1280]
assert sum(USPLIT) == GEO["bf16u"]["BS"]
for g in GEO.values():
    g["SW"] = g["NBUCK"] * g["BS"]
    assert sum(g["BLOCKS"]) == g["BS"]

f32 = mybir.dt.float32
bf16 = mybir.dt.bfloat16
i16 = mybir.dt.int16

_NC_CACHE = {}


def build_nc(mode=MODE, loop_k=None):
    NBUCK = GEO[mode]["NBUCK"]
    BS = GEO[mode]["BS"]
    BLOCKS = GEO[mode]["BLOCKS"]
    SW = GEO[mode]["SW"]
    BROWS = NROWS // NBUCK
    HB = NBUCK if mode == "bf16u" else KH * NBUCK
    TILES = BS // P

    nc = bacc.Bacc("TRN2", target_bir_lowering=False, debug=False,
                   num_swdge_queues=4)
    if mode == "fp32":
        feat = nc.dram_tensor("feat", [NROWS, D], f32, kind="ExternalInput")
        w = nc.dram_tensor("w", [P, KH * 2 * V], f32, kind="ExternalInput")
    else:
        feat = nc.dram_tensor("feat", [NROWS, D], bf16, kind="ExternalInput")
        w = nc.dram_tensor("w", [P, KH * 2 * V], bf16, kind="ExternalInput")
        # bias folded into a PE matmul: row 0 of biasw carries the bias,
        # rows 1..127 are zero, against an all-ones rhs — full 128-partition
        # contraction (1-partition matmuls misbehave on HW)
        biasw = nc.dram_tensor("biasw", [P, KH * V], bf16, kind="ExternalInput")
        ones = nc.dram_tensor("ones", [P, 512], bf16, kind="ExternalInput")

    idx = nc.dram_tensor("idx", [HB, P, BS // 16], i16, kind="ExternalInput")
    if mode == "fp32":
        bias = nc.dram_tensor("bias", [V, KH], f32, kind="ExternalInput")
    if mode == "bf16u":
        out = nc.dram_tensor("out", [KH * V, SW], f32, kind="ExternalOutput")
    else:
        out = nc.dram_tensor("out", [KH, V, SW], f32, kind="ExternalOutput")

    with tile.TileContext(nc) as tc:
        with tc.tile_pool(name="const", bufs=1) as cpool, \
             tc.tile_pool(name="gath", bufs=3) as gpool, \
             tc.tile_pool(name="gt", bufs=4) as gtpool, \
             tc.tile_pool(name="ob", bufs=2) as obpool, \
             tc.tile_pool(name="pst", bufs=3, space="PSUM") as ptpool, \
             tc.tile_pool(name="pso", bufs=4, space="PSUM") as popool:

            w_sb = cpool.tile(list(w.shape), w.dtype)
            nc.sync.dma_start(w_sb[:], w[:])
            idx_sb = cpool.tile([P, HB, BS // 16], i16)
            for hb in range(HB):
                nc.sync.dma_start(idx_sb[:, hb, :], idx[hb])
            ident = None
            if mode == "fp32":
                bias_sb = cpool.tile([V, KH], f32)
                nc.sync.dma_start(bias_sb[:], bias[:])
                ident = cpool.tile([P, P], f32)
                make_identity(nc, ident[:])
            else:
                biasw_sb = cpool.tile([P, KH * V], bf16)
                nc.sync.dma_start(biasw_sb[:], biasw[:])
                ones_sb = cpool.tile([P, 512], bf16)
                nc.sync.dma_start(ones_sb[:], ones[:])
                if mode in ("bf16n", "bf16u"):
                    ident = cpool.tile([P, P], bf16)
                    make_identity(nc, ident[:])

            import contextlib
            loop_cm = tc.For_i(0, loop_k, 1) if loop_k else contextlib.nullcontext()
            with loop_cm:
                KHV = KH * V
                for b in range(NBUCK) if mode == "bf16u" else ():
                    g = gpool.tile([P, TILES, D], bf16, tag="g")
                    toff = 0
                    for q, NQ in enumerate(USPLIT):
                        nc.gpsimd.dma_gather(
                            g[:, toff // P:(toff + NQ) // P, :],
                            feat[b * BROWS:(b + 1) * BROWS, :],
                            idx_sb[:, b, toff // 16:(toff + NQ) // 16],
                            NQ, NQ, D, single_packet=False, queue_num=q)
                        toff += NQ
                    ob = obpool.tile([KHV, BS], f32, tag="ob")
                    off = 0
                    for blk, NW in enumerate(BLOCKS):
                        pt = ptpool.tile([P, 1024], bf16, tag="pt")
                        for tl in range(NW // P):
                            t = off // P + tl
                            nc.tensor.transpose(
                                out=pt[:, tl * P:(tl + 1) * P],
                                in_=g[:, t, 0:P], identity=ident[:])
                            nc.tensor.transpose(
                                out=pt[:, 512 + tl * P:512 + (tl + 1) * P],
                                in_=g[:, t, P:D], identity=ident[:])
                        gt = gtpool.tile([P, 1024], bf16, tag="gt")
                        nc.vector.tensor_copy(gt[:, :NW], pt[:, :NW])
                        nc.scalar.activation(
                            gt[:, 512:512 + NW], pt[:, 512:512 + NW],
                            mybir.ActivationFunctionType.Identity)
                        po = popool.tile([KHV, 512], f32, tag="po")
                        nc.tensor.matmul(
                            po[:, :NW], lhsT=w_sb[:, 0:KHV],
                            rhs=gt[:, :NW], start=True, stop=False)
                        nc.tensor.matmul(
                            po[:, :NW], lhsT=w_sb[:, KHV:2 * KHV],
                            rhs=gt[:, 512:512 + NW], start=False, stop=False)
                        nc.tensor.matmul(
                            po[:, :NW], lhsT=biasw_sb[:, 0:KHV],
                            rhs=ones_sb[:, :NW], start=False, stop=True)
                        if blk % 2 == 0:
                            nc.vector.tensor_copy(ob[:, off:off + NW],
                                                  po[:, :NW])
                        else:
                            nc.scalar.activation(
                                ob[:, off:off + NW], po[:KHV, :NW],
                                mybir.ActivationFunctionType.Identity)
                        off += NW
                    nc.sync.dma_start(out[:, b * BS:(b + 1) * BS], ob[:])
                for h in range(KH) if mode != "bf16u" else ():
                    for b in range(NBUCK):
                        hb = h * NBUCK + b
                        if mode == "fp32":
                            g = gpool.tile([P, TILES, D], f32, tag="g")
                            # split across all 4 SWDGE queues for more
                            # outstanding HBM reads
                            HT = TILES // 4
                            HN = BS // 4
                            for q in range(4):
                                nc.gpsimd.dma_gather(
                                    g[:, q * HT:(q + 1) * HT, :],
                                    feat[b * BROWS:(b + 1) * BROWS, :],
                                    idx_sb[:, hb, q * (HN // 16):(q + 1) * (HN // 16)],
                                    HN, HN, D,
                                    single_packet=False, queue_num=q)
                        elif mode == "bf16n":
                            # non-transpose bf16 gather, 4 queues (multi-queue
                            # is HW-safe for non-transpose gathers), then PE
                            # transposes each [128,128] tile (1 cyc/row bf16)
                            g = gpool.tile([P, TILES, D], bf16, tag="g")
                            toff = 0
                            for q, NQ in enumerate(NSPLIT):
                                nc.gpsimd.dma_gather(
                                    g[:, toff // P:(toff + NQ) // P, :],
                                    feat[b * BROWS:(b + 1) * BROWS, :],
                                    idx_sb[:, hb, toff // 16:(toff + NQ) // 16],
                                    NQ, NQ, D,
                                    single_packet=False, queue_num=q)
                                toff += NQ
                            ob = obpool.tile([V, BS], f32, tag="ob")
                            off = 0
                            for blk, NW in enumerate(BLOCKS):
                                # both 128-row D chunks transposed into one
                                # PSUM bank: [:, 0:512] = D 0:128,
                                # [:, 512:1024] = D 128:256
                                pt = ptpool.tile([P, 1024], bf16, tag="pt")
                                for tl in range(NW // P):
                                    t = off // P + tl
                                    nc.tensor.transpose(
                                        out=pt[:, tl * P:(tl + 1) * P],
                                        in_=g[:, t, 0:P], identity=ident[:])
                                    nc.tensor.transpose(
                                        out=pt[:, 512 + tl * P:512 + (tl + 1) * P],
                                        in_=g[:, t, P:D], identity=ident[:])
                                gt = gtpool.tile([P, 1024], bf16, tag="gt")
                                nc.vector.tensor_copy(gt[:, :NW], pt[:, :NW])
                                nc.scalar.activation(
                                    gt[:, 512:512 + NW], pt[:, 512:512 + NW],
                                    mybir.ActivationFunctionType.Identity)
                                po = popool.tile([V, 512], f32, tag="po")
                                nc.tensor.matmul(
                                    po[:, :NW],
                                    lhsT=w_sb[:, (h * 2 + 0) * V:(h * 2 + 1) * V],
                                    rhs=gt[:, :NW], start=True, stop=False)
                                nc.tensor.matmul(
                                    po[:, :NW],
                                    lhsT=w_sb[:, (h * 2 + 1) * V:(h * 2 + 2) * V],
                                    rhs=gt[:, 512:512 + NW], start=False, stop=False)
                                nc.tensor.matmul(
                                    po[:, :NW],
                                    lhsT=biasw_sb[:, h * V:(h + 1) * V],
                                    rhs=ones_sb[:, :NW],
                                    start=False, stop=True)
                                if blk % 2 == 0:
                                    nc.vector.tensor_copy(
                                        ob[:, off:off + NW], po[:, :NW])
                                else:
                                    nc.scalar.activation(
                                        ob[:, off:off + NW], po[:V, :NW],
                                        mybir.ActivationFunctionType.Identity)
                                off += NW
                            nc.sync.dma_start(out[h, :, b * BS:(b + 1) * BS],
                                              ob[:])
                            continue
                        else:
                            # split across all 4 SWDGE queues, same queue set
                            # every bucket: queue-FIFO keeps cross-bucket
                            # completion ordered (one gather per bucket on
                            # rotating queues raced on HW). One tile per
                            # queue chunk (transpose gather needs contiguous
                            # free dims), so matmuls start per-chunk.
                            gs = []
                            off_q = 0
                            for q, NQ in enumerate(QSPLIT):
                                gq = gpool.tile([P, 2, NQ], bf16, tag=f"g{q}")
                                nc.gpsimd.dma_gather(
                                    gq[:], feat[b * BROWS:(b + 1) * BROWS, :],
                                    idx_sb[:, hb, off_q // 16:(off_q + NQ) // 16],
                                    NQ, NQ, D,
                                    transpose=True, single_packet=False,
                                    queue_num=0)
                                gs.append((gq, off_q, NQ))
                                off_q += NQ
                            ob = obpool.tile([V, BS], f32, tag="ob")
                            blk = 0
                            for gq, qoff, NQ in gs:
                                loff = 0
                                while loff < NQ:
                                    NW = min(512, NQ - loff)
                                    sl = slice(loff, loff + NW)
                                    po = popool.tile([V, 512], f32, tag="po")
                                    nc.tensor.matmul(
                                        po[:, :NW],
                                        lhsT=w_sb[:, (h * 2 + 0) * V:(h * 2 + 1) * V],
                                        rhs=gq[:, 0, sl], start=True, stop=False)
                                    nc.tensor.matmul(
                                        po[:, :NW],
                                        lhsT=w_sb[:, (h * 2 + 1) * V:(h * 2 + 2) * V],
                                        rhs=gq[:, 1, sl], start=False, stop=False)
                                    nc.tensor.matmul(
                                        po[:, :NW],
                                        lhsT=biasw_sb[:, h * V:(h + 1) * V],
                                        rhs=ones_sb[:, :NW],
                                        start=False, stop=True)
                                    osl = slice(qoff + loff, qoff + loff + NW)
                                    # alternate PSUM->SBUF copy between the
                                    # two free engines
                                    if blk % 2 == 0:
                                        nc.vector.tensor_copy(ob[:, osl], po[:, :NW])
                                    else:
                                        nc.scalar.activation(
                                            ob[:, osl], po[:V, :NW],
                                            mybir.ActivationFunctionType.Identity)
                                    blk += 1
                                    loff += NW
                            nc.sync.dma_start(out[h, :, b * BS:(b + 1) * BS],
                                              ob[:])
                            continue
                        off = 0
                        for blk, NW in enumerate(BLOCKS):
                            if mode == "fp32":
                                pt0 = ptpool.tile([P, 512], f32, tag="pt0")
                                pt1 = ptpool.tile([P, 512], f32, tag="pt1")
                                for tl in range(NW // P):
                                    t = off // P + tl
                                    nc.tensor.transpose(
                                        out=pt0[:, tl * P:(tl + 1) * P],
                                        in_=g[:, t, 0:P], identity=ident[:])
                                    nc.tensor.transpose(
                                        out=pt1[:, tl * P:(tl + 1) * P],
                                        in_=g[:, t, P:D], identity=ident[:])
                                gt0 = gtpool.tile([P, 512], f32, tag="gt0")
                                gt1 = gtpool.tile([P, 512], f32, tag="gt1")
                                nc.vector.tensor_copy(gt0[:, :NW], pt0[:, :NW])
                                nc.vector.tensor_copy(gt1[:, :NW], pt1[:, :NW])
                                po = popool.tile([V, 512], f32, tag="po")
                                nc.tensor.matmul(
                                    po[:, :NW],
                                    lhsT=w_sb[:, (h * 2 + 0) * V:(h * 2 + 1) * V],
                                    rhs=gt0[:, :NW], start=True, stop=False)
                                nc.tensor.matmul(
                                    po[:, :NW],
                                    lhsT=w_sb[:, (h * 2 + 1) * V:(h * 2 + 2) * V],
                                    rhs=gt1[:, :NW], start=False, stop=True)
                                ob = obpool.tile([V, 512], f32, tag="ob")
                                nc.scalar.activation(
                                    ob[:, :NW], po[:V, :NW],
                                    mybir.ActivationFunctionType.Identity,
                                    bias=bias_sb[:, h:h + 1])
                                nc.sync.dma_start(
                                    out[h, :, b * BS + off: b * BS + off + NW],
                                    ob[:, :NW])
                            off += NW
    nc.compile()
    return nc


def get_nc(mode=MODE):
    if mode not in _NC_CACHE:
        _NC_CACHE[mode] = build_nc(mode)
    return _NC_CACHE[mode]


def _wrap_idx(a, BS):
    """[BS] int16 -> [P, BS//16]: idx i at [i % 16, i // 16], replicated x8."""
    return np.tile(a.reshape(BS // 16, 16).T, (8, 1))


def prep_inputs(features, mask_idx, head_weights, head_bias, mode=MODE):
    """Build per-core in_maps + the unpermute info."""
    NBUCK = GEO[mode]["NBUCK"]
    BS = GEO[mode]["BS"]
    HB = KH * NBUCK
    shift = {4: 15, 8: 14}[NBUCK]
    mask = (1 << shift) - 1

    feats = np.ascontiguousarray(
        np.asarray(features, dtype=np.float32).reshape(NROWS, D))
    mask_idx = np.asarray(mask_idx, dtype=np.int32)
    W = np.asarray(head_weights, dtype=np.float32)
    hbias = np.asarray(head_bias, dtype=np.float32)

    if mode == "fp32":
        feat_in = feats
    else:
        import ml_dtypes
        feat_in = feats.astype(ml_dtypes.bfloat16)

    if mode == "bf16u":
        import ml_dtypes
        in_maps, unperm = [], []   # unperm[k] = device column per position
        for c in range(NCORES):
            allr = mask_idx[c * KH:(c + 1) * KH].ravel()
            idx_payload = np.zeros((NBUCK, P, BS // 16), np.int16)
            ulist = []
            for bb in range(NBUCK):
                u = np.unique(allr[(allr >> shift) == bb])
                assert u.size <= BS, f"union bucket overflow: {u.size}"
                padded = np.zeros(BS, np.int16)
                padded[:u.size] = (u & mask).astype(np.int16)
                idx_payload[bb] = _wrap_idx(padded, BS)
                ulist.append(u)
            for hidx in range(KH):
                gid = mask_idx[c * KH + hidx]
                cols = np.empty(S, np.int64)
                bb_all = gid >> shift
                for bb in range(NBUCK):
                    m = bb_all == bb
                    cols[m] = bb * BS + np.searchsorted(ulist[bb], gid[m])
                unperm.append(cols)
            Wc = W[c * KH:(c + 1) * KH]
            w_in = np.ascontiguousarray(
                Wc.reshape(KH, 2, P, V).transpose(2, 1, 0, 3).reshape(P, 2 * KH * V))
            biasw_in = np.zeros((P, KH * V), np.float32)
            biasw_in[0] = hbias[c * KH:(c + 1) * KH].ravel()
            in_maps.append({
                "feat": feat_in, "idx": idx_payload,
                "w": w_in.astype(ml_dtypes.bfloat16),
                "biasw": biasw_in.astype(ml_dtypes.bfloat16),
                "ones": np.ones((P, 512), ml_dtypes.bfloat16),
            })
        return in_maps, unperm

    in_maps = []
    unperm = []   # per head: (order, counts)
    for c in range(NCORES):
        idx_payload = np.zeros((HB, P, BS // 16), np.int16)
        for hidx, k in enumerate(range(c * KH, (c + 1) * KH)):
            gid = mask_idx[k]
            bidx = gid >> shift
            # sort by full index (not just bucket id): within-bucket ascending
            # order gives the SDMA engines HBM-page-local read streams
            order = np.argsort(gid, kind="stable")
            counts = np.bincount(bidx, minlength=NBUCK)
            assert counts.max() <= BS, f"bucket overflow: {counts}"
            rel = (gid & mask).astype(np.int16)
            pos = 0
            for bb in range(NBUCK):
                cnt = int(counts[bb])
                padded = np.zeros(BS, np.int16)
                padded[:cnt] = rel[order[pos:pos + cnt]]
                idx_payload[hidx * NBUCK + bb] = _wrap_idx(padded, BS)
                pos += cnt
            unperm.append((order, counts))

        Wc = W[c * KH:(c + 1) * KH]          # [KH, 256, 8]
        w_in = np.ascontiguousarray(
            Wc.reshape(KH, 2, P, V).transpose(2, 0, 1, 3).reshape(P, KH * 2 * V))
        if mode == "fp32":
            bias_in = np.ascontiguousarray(hbias[c * KH:(c + 1) * KH].T)  # [V, KH]
            in_maps.append({"feat": feat_in, "idx": idx_payload,
                            "w": w_in, "bias": bias_in})
        else:
            import ml_dtypes
            biasw_in = np.zeros((P, KH * V), np.float32)
            biasw_in[0] = hbias[c * KH:(c + 1) * KH].reshape(KH * V)
            in_maps.append({
                "feat": feat_in, "idx": idx_payload,
                "w": w_in.astype(ml_dtypes.bfloat16),
                "biasw": biasw_in.astype(ml_dtypes.bfloat16),
                "ones": np.ones((P, 512), ml_dtypes.bfloat16),
            })
    return in_maps, unperm


def assemble_output(results, unperm, mode=MODE):
    NBUCK = GEO[mode]["NBUCK"]
    BS = GEO[mode]["BS"]
    out_full = np.empty((NCORES * KH, S, V), np.float32)
    for c in range(NCORES):
        dev = results[c]["out"]              # [KH, V, SW] or [KH*V, SW]
        for h in range(KH):
            k = c * KH + h
            if mode == "bf16u":
                out_full[k] = dev[h * V:(h + 1) * V][:, unperm[k]].T
                continue
            order, counts = unperm[k]
            cols = np.concatenate(
                [np.arange(bb * BS, bb * BS + counts[bb]) for bb in range(NBUCK)])
            out_full[k, order, :] = dev[h][:, cols].T
    return out_full


def kernel(block_type_grid=None, features=None, mask_idx=None,
           head_weights=None, head_bias=None):
    nc = get_nc(MODE)
    in_maps, unperm = prep_inputs(features, mask_idx, head_weights, head_bias, MODE)
    res = run_bass_kernel_spmd(nc, in_maps, list(range(NCORES)))
    return assemble_output(res.results, unperm, MODE)


# revision 32
# speedup vs baseline: 3.9288x; 1.0135x over previous
"""Trainium2 Bass kernel for nn_AttributeDecoder (gather + per-head small linear).

  logits[k, s, v] = features.reshape(-1, 256)[mask_idx[k, s], :] @ W[k] + b[k]
  K=24 heads, S=16384 positions/head, D=256, V=8, N=131072 table rows.

Sharding: expert-parallel over heads — 3 heads per core x 8 cores, features
table replicated (per-core DRAM copy, gathered via dma_gather).

Per (core, head): indices bucketed by table region (so in-bucket offsets fit
dma_gather's int16 indices), padded to a fixed bucket size.

bf16u mode (default): features stored bf16 (512B rows — half the gather
traffic of fp32). Per bucket, gather the UNION of the core's 3 heads' rows
(~17% fewer rows than per-head gathers); for each gathered row compute all
24 logits in one PE pass (lhsT [128, 24] costs the same as [128, 8]); the
host expands union positions back to per-head order. Non-transpose gather
split over all 4 SWDGE queues (multi-queue is only sync-safe for
non-transpose gathers, and each DMASW completion lane must always map to
one queue), PE transposes each [128,128] bf16 tile (1 cyc/row), bias folds
into a third matmul against an all-ones rhs, PSUM->SBUF copies alternate
Vector/Scalar engines, one batched DMA out per bucket.
absmax err ~2.3e-3 relative, well under the 2e-2 gate.

HW-validated lessons encoded here:
  - transpose=True dma_gather corrupts data under multi-queue (completion
    signaling races); single-queue is correct but ~3x slower.
  - each SWDGE queue sustains only ~45-50 GB/s; 4 queues are needed to
    approach the ~180 GB/s/core gather ceiling.
  - 1-partition matmuls (contraction dim 1) misbehave on HW; bias matmul
    uses a full 128-partition lhsT with zeros in rows 1..127.

Other modes kept for reference: bf16n (per-head gathers), bf16t
(transposed gather, single queue), fp32 (exact, original baseline).

Host unpermutes the bucketed output order.
"""
import os
import numpy as np

import concourse.bass as bass
import concourse.mybir as mybir
import concourse.tile as tile
from concourse import bacc
from concourse.bass_utils import run_bass_kernel_spmd
from concourse.masks import make_identity

NCORES = 8
KH = 3                 # heads per core
P = 128
D = 256
V = 8
S = 16384
NROWS = 131072

MODE = os.environ.get("KERNEL_MODE", "bf16u")   # bf16u|bf16n|bf16t|fp32

# per-mode gather geometry
GEO = {
    # NBUCK buckets of NROWS/NBUCK rows; BS padded positions per bucket;
    # BLOCKS: matmul n-block widths covering BS positions
    "fp32":  dict(NBUCK=4, BS=4608, BLOCKS=[512] * 9),
    "bf16t": dict(NBUCK=4, BS=4352, BLOCKS=[512] * 8 + [256]),
    "bf16n": dict(NBUCK=4, BS=4352, BLOCKS=[512] * 8 + [256]),
    # bf16u: per-bucket gather of the UNION of the core's 3 heads' rows
    # (~17% fewer rows), all 24 logits computed per gathered row
    "bf16u": dict(NBUCK=8, BS=5248, BLOCKS=[512] * 10 + [128]),
}
# sub-gather sizes for bf16t: multiples of 512 so matmul blocks never span
# chunk tiles (each sub-gather writes its own contiguous tile). All gathers
# MUST share one SWDGE queue: concurrent queues complete out of order and
# break the tile framework's cumulative DMA-completion semaphores (observed
# as block-granular garbage on HW).
QSPLIT = [4352]
assert sum(QSPLIT) == GEO["bf16t"]["BS"]
# bf16n non-transpose sub-gathers: 4 queues (multi-queue is safe for
# NON-transpose gathers — the fp32 baseline proved it on HW — and is 3x
# faster than one queue since each queue feeds its own DMA rings).
# Sizes are multiples of 128 so each sub-gather covers whole 128-row tiles.
NSPLIT = [1152, 1024, 1152, 1024]
assert sum(NSPLIT) == GEO["bf16n"]["BS"]
USPLIT = [1408, 1280, 1280, 1280]
assert sum(USPLIT) == GEO["bf16u"]["BS"]
for g in GEO.values():
    g["SW"] = g["NBUCK"] * g["BS"]
    assert sum(g["BLOCKS"]) == g["BS"]

f32 = mybir.dt.float32
bf16 = mybir.dt.bfloat16
i16 = mybir.dt.int16

_NC_CACHE = {}


def build_nc(mode=MODE, loop_k=None):
    NBUCK = GEO[mode]["NBUCK"]
    BS = GEO[mode]["BS"]
    BLOCKS = GEO[mode]["BLOCKS"]
    SW = GEO[mode]["SW"]
    BROWS = NROWS // NBUCK
    HB = NBUCK if mode == "bf16u" else KH * NBUCK
    TILES = BS // P

    nc = bacc.Bacc("TRN2", target_bir_lowering=False, debug=False,
                   num_swdge_queues=4)
    if mode == "fp32":
        feat = nc.dram_tensor("feat", [NROWS, D], f32, kind="ExternalInput")
        w = nc.dram_tensor("w", [P, KH * 2 * V], f32, kind="ExternalInput")
    else:
        feat = nc.dram_tensor("feat", [NROWS, D], bf16, kind="ExternalInput")
        w = nc.dram_tensor("w", [P, KH * 2 * V], bf16, kind="ExternalInput")
        # bias folded into a PE matmul: row 0 of biasw carries the bias,
        # rows 1..127 are zero, against an all-ones rhs — full 128-partition
        # contraction (1-partition matmuls misbehave on HW)
        biasw = nc.dram_tensor("biasw", [P, KH * V], bf16, kind="ExternalInput")
        ones = nc.dram_tensor("ones", [P, 512], bf16, kind="ExternalInput")

    idx = nc.dram_tensor("idx", [HB, P, BS // 16], i16, kind="ExternalInput")
    if mode == "fp32":
        bias = nc.dram_tensor("bias", [V, KH], f32, kind="ExternalInput")
    if mode == "bf16u":
        out = nc.dram_tensor("out", [KH * V, SW], f32, kind="ExternalOutput")
    else:
        out = nc.dram_tensor("out", [KH, V, SW], f32, kind="ExternalOutput")

    with tile.TileContext(nc) as tc:
        with tc.tile_pool(name="const", bufs=1) as cpool, \
             tc.tile_pool(name="gath", bufs=4) as gpool, \
             tc.tile_pool(name="gt", bufs=4) as gtpool, \
             tc.tile_pool(name="ob", bufs=2) as obpool, \
             tc.tile_pool(name="pst", bufs=3, space="PSUM") as ptpool, \
             tc.tile_pool(name="pso", bufs=4, space="PSUM") as popool:

            w_sb = cpool.tile(list(w.shape), w.dtype)
            nc.sync.dma_start(w_sb[:], w[:])
            idx_sb = cpool.tile([P, HB, BS // 16], i16)
            for hb in range(HB):
                nc.sync.dma_start(idx_sb[:, hb, :], idx[hb])
            ident = None
            if mode == "fp32":
                bias_sb = cpool.tile([V, KH], f32)
                nc.sync.dma_start(bias_sb[:], bias[:])
                ident = cpool.tile([P, P], f32)
                make_identity(nc, ident[:])
            else:
                biasw_sb = cpool.tile([P, KH * V], bf16)
                nc.sync.dma_start(biasw_sb[:], biasw[:])
                ones_sb = cpool.tile([P, 512], bf16)
                nc.sync.dma_start(ones_sb[:], ones[:])
                if mode in ("bf16n", "bf16u"):
                    ident = cpool.tile([P, P], bf16)
                    make_identity(nc, ident[:])

            import contextlib
            loop_cm = tc.For_i(0, loop_k, 1) if loop_k else contextlib.nullcontext()
            with loop_cm:
                KHV = KH * V
                for b in range(NBUCK) if mode == "bf16u" else ():
                    g = gpool.tile([P, TILES, D], bf16, tag="g")
                    toff = 0
                    # fixed queue per sub-slot: the DMASW completion lanes
                    # (issue order % 8) must each always map to one queue,
                    # or cross-queue completions race the lane semaphores
                    for i, NQ in enumerate(USPLIT):
                        q = i
                        nc.gpsimd.dma_gather(
                            g[:, toff // P:(toff + NQ) // P, :],
                            feat[b * BROWS:(b + 1) * BROWS, :],
                            idx_sb[:, b, toff // 16:(toff + NQ) // 16],
                            NQ, NQ, D, single_packet=False, queue_num=q)
                        toff += NQ
                    ob = obpool.tile([KHV, BS], f32, tag="ob")
                    off = 0
                    for blk, NW in enumerate(BLOCKS):
                        pt = ptpool.tile([P, 1024], bf16, tag="pt")
                        for tl in range(NW // P):
                            t = off // P + tl
                            nc.tensor.transpose(
                                out=pt[:, tl * P:(tl + 1) * P],
                                in_=g[:, t, 0:P], identity=ident[:])
                            nc.tensor.transpose(
                                out=pt[:, 512 + tl * P:512 + (tl + 1) * P],
                                in_=g[:, t, P:D], identity=ident[:])
                        gt = gtpool.tile([P, 1024], bf16, tag="gt")
                        nc.vector.tensor_copy(gt[:, :NW], pt[:, :NW])
                        nc.scalar.activation(
                            gt[:, 512:512 + NW], pt[:, 512:512 + NW],
                            mybir.ActivationFunctionType.Identity)
                        po = popool.tile([KHV, 512], f32, tag="po")
                        nc.tensor.matmul(
                            po[:, :NW], lhsT=w_sb[:, 0:KHV],
                            rhs=gt[:, :NW], start=True, stop=False)
                        nc.tensor.matmul(
                            po[:, :NW], lhsT=w_sb[:, KHV:2 * KHV],
                            rhs=gt[:, 512:512 + NW], start=False, stop=False)
                        nc.tensor.matmul(
                            po[:, :NW], lhsT=biasw_sb[:, 0:KHV],
                            rhs=ones_sb[:, :NW], start=False, stop=True)
                        if blk % 2 == 0:
                            nc.vector.tensor_copy(ob[:, off:off + NW],
                                                  po[:, :NW])
                        else:
                            nc.scalar.activation(
                                ob[:, off:off + NW], po[:KHV, :NW],
                                mybir.ActivationFunctionType.Identity)
                        off += NW
                    nc.sync.dma_start(out[:, b * BS:(b + 1) * BS], ob[:])
                for h in range(KH) if mode != "bf16u" else ():
                    for b in range(NBUCK):
                        hb = h * NBUCK + b
                        if mode == "fp32":
                            g = gpool.tile([P, TILES, D], f32, tag="g")
                            # split across all 4 SWDGE queues for more
                            # outstanding HBM reads
                            HT = TILES // 4
                            HN = BS // 4
                            for q in range(4):
                                nc.gpsimd.dma_gather(
                                    g[:, q * HT:(q + 1) * HT, :],
                                    feat[b * BROWS:(b + 1) * BROWS, :],
                                    idx_sb[:, hb, q * (HN // 16):(q + 1) * (HN // 16)],
                                    HN, HN, D,
                                    single_packet=False, queue_num=q)
                        elif mode == "bf16n":
                            # non-transpose bf16 gather, 4 queues (multi-queue
                            # is HW-safe for non-transpose gathers), then PE
                            # transposes each [128,128] tile (1 cyc/row bf16)
                            g = gpool.tile([P, TILES, D], bf16, tag="g")
                            toff = 0
                            for q, NQ in enumerate(NSPLIT):
                                nc.gpsimd.dma_gather(
                                    g[:, toff // P:(toff + NQ) // P, :],
                                    feat[b * BROWS:(b + 1) * BROWS, :],
                                    idx_sb[:, hb, toff // 16:(toff + NQ) // 16],
                                    NQ, NQ, D,
                                    single_packet=False, queue_num=q)
                                toff += NQ
                            ob = obpool.tile([V, BS], f32, tag="ob")
                            off = 0
                            for blk, NW in enumerate(BLOCKS):
                                # both 128-row D chunks transposed into one
                                # PSUM bank: [:, 0:512] = D 0:128,
                                # [:, 512:1024] = D 128:256
                                pt = ptpool.tile([P, 1024], bf16, tag="pt")
                                for tl in range(NW // P):
                                    t = off // P + tl
                                    nc.tensor.transpose(
                                        out=pt[:, tl * P:(tl + 1) * P],
                                        in_=g[:, t, 0:P], identity=ident[:])
                                    nc.tensor.transpose(
                                        out=pt[:, 512 + tl * P:512 + (tl + 1) * P],
                                        in_=g[:, t, P:D], identity=ident[:])
                                gt = gtpool.tile([P, 1024], bf16, tag="gt")
                                nc.vector.tensor_copy(gt[:, :NW], pt[:, :NW])
                                nc.scalar.activation(
                                    gt[:, 512:512 + NW], pt[:, 512:512 + NW],
                                    mybir.ActivationFunctionType.Identity)
                                po = popool.tile([V, 512], f32, tag="po")
                                nc.tensor.matmul(
                                    po[:, :NW],
                                    lhsT=w_sb[:, (h * 2 + 0) * V:(h * 2 + 1) * V],
                                    rhs=gt[:, :NW], start=True, stop=False)
                                nc.tensor.matmul(
                                    po[:, :NW],
                                    lhsT=w_sb[:, (h * 2 + 1) * V:(h * 2 + 2) * V],
                                    rhs=gt[:, 512:512 + NW], start=False, stop=False)
                                nc.tensor.matmul(
                                    po[:, :NW],
                                    lhsT=biasw_sb[:, h * V:(h + 1) * V],
                                    rhs=ones_sb[:, :NW],
                                    start=False, stop=True)
                                if blk % 2 == 0:
                                    nc.vector.tensor_copy(
                                        ob[:, off:off + NW], po[:, :NW])
                                else:
                                    nc.scalar.activation(
                                        ob[:, off:off + NW], po[:V, :NW],
                                        mybir.ActivationFunctionType.Identity)
                                off += NW
                            nc.sync.dma_start(out[h, :, b * BS:(b + 1) * BS],
                                              ob[:])
                            continue
                        else:
                            # split across all 4 SWDGE queues, same queue set
                            # every bucket: queue-FIFO keeps cross-bucket
                            # completion ordered (one gather per bucket on
                            # rotating queues raced on HW). One tile per
                            # queue chunk (transpose gather needs contiguous
                            # free dims), so matmuls start per-chunk.
                            gs = []
                            off_q = 0
                            for q, NQ in enumerate(QSPLIT):
                                gq = gpool.tile([P, 2, NQ], bf16, tag=f"g{q}")
                                nc.gpsimd.dma_gather(
                                    gq[:], feat[b * BROWS:(b + 1) * BROWS, :],
                                    idx_sb[:, hb, off_q // 16:(off_q + NQ) // 16],
                                    NQ, NQ, D,
                                    transpose=True, single_packet=False,
                                    queue_num=0)
                                gs.append((gq, off_q, NQ))
                                off_q += NQ
                            ob = obpool.tile([V, BS], f32, tag="ob")
                            blk = 0
                            for gq, qoff, NQ in gs:
                                loff = 0
                                while loff < NQ:
                                    NW = min(512, NQ - loff)
                                    sl = slice(loff, loff + NW)
                                    po = popool.tile([V, 512], f32, tag="po")
                                    nc.tensor.matmul(
                                        po[:, :NW],
                                        lhsT=w_sb[:, (h * 2 + 0) * V:(h * 2 + 1) * V],
                                        rhs=gq[:, 0, sl], start=True, stop=False)
                                    nc.tensor.matmul(
                                        po[:, :NW],
                                        lhsT=w_sb[:, (h * 2 + 1) * V:(h * 2 + 2) * V],
                                        rhs=gq[:, 1, sl], start=False, stop=False)
                                    nc.tensor.matmul(
                                        po[:, :NW],
                                        lhsT=biasw_sb[:, h * V:(h + 1) * V],
                                        rhs=ones_sb[:, :NW],
                                        start=False, stop=True)
                                    osl = slice(qoff + loff, qoff + loff + NW)
                                    # alternate PSUM->SBUF copy between the
                                    # two free engines
                                    if blk % 2 == 0:
                                        nc.vector.tensor_copy(ob[:, osl], po[:, :NW])
                                    else:
                                        nc.scalar.activation(
                                            ob[:, osl], po[:V, :NW],
                                            mybir.ActivationFunctionType.Identity)
                                    blk += 1
                                    loff += NW
                            nc.sync.dma_start(out[h, :, b * BS:(b + 1) * BS],
                                              ob[:])
                            continue
                        off = 0
                        for blk, NW in enumerate(BLOCKS):
                            if mode == "fp32":
                                pt0 = ptpool.tile([P, 512], f32, tag="pt0")
                                pt1 = ptpool.tile([P, 512], f32, tag="pt1")
                                for tl in range(NW // P):
                                    t = off // P + tl
                                    nc.tensor.transpose(
                                        out=pt0[:, tl * P:(tl + 1) * P],
                                        in_=g[:, t, 0:P], identity=ident[:])
                                    nc.tensor.transpose(
                                        out=pt1[:, tl * P:(tl + 1) * P],
                                        in_=g[:, t, P:D], identity=ident[:])
                                gt0 = gtpool.tile([P, 512], f32, tag="gt0")
                                gt1 = gtpool.tile([P, 512], f32, tag="gt1")
                                nc.vector.tensor_copy(gt0[:, :NW], pt0[:, :NW])
                                nc.vector.tensor_copy(gt1[:, :NW], pt1[:, :NW])
                                po = popool.tile([V, 512], f32, tag="po")
                                nc.tensor.matmul(
                                    po[:, :NW],
                                    lhsT=w_sb[:, (h * 2 + 0) * V:(h * 2 + 1) * V],
                                    rhs=gt0[:, :NW], start=True, stop=False)
                                nc.tensor.matmul(
                                    po[:, :NW],
                                    lhsT=w_sb[:, (h * 2 + 1) * V:(h * 2 + 2) * V],
                                    rhs=gt1[:, :NW], start=False, stop=True)
                                ob = obpool.tile([V, 512], f32, tag="ob")
                                nc.scalar.activation(
                                    ob[:, :NW], po[:V, :NW],
                                    mybir.ActivationFunctionType.Identity,
                                    bias=bias_sb[:, h:h + 1])
                                nc.sync.dma_start(
                                    out[h, :, b * BS + off: b * BS + off + NW],
                                    ob[:, :NW])
                            off += NW
    nc.compile()
    return nc


def get_nc(mode=MODE):
    if mode not in _NC_CACHE:
        _NC_CACHE[mode] = build_nc(mode)
    return _NC_CACHE[mode]


def _wrap_idx(a, BS):
    """[BS] int16 -> [P, BS//16]: idx i at [i % 16, i // 16], replicated x8."""
    return np.tile(a.reshape(BS // 16, 16).T, (8, 1))


def prep_inputs(features, mask_idx, head_weights, head_bias, mode=MODE):
    """Build per-core in_maps + the unpermute info."""
    NBUCK = GEO[mode]["NBUCK"]
    BS = GEO[mode]["BS"]
    HB = KH * NBUCK
    shift = {4: 15, 8: 14}[NBUCK]
    mask = (1 << shift) - 1

    feats = np.ascontiguousarray(
        np.asarray(features, dtype=np.float32).reshape(NROWS, D))
    mask_idx = np.asarray(mask_idx, dtype=np.int32)
    W = np.asarray(head_weights, dtype=np.float32)
    hbias = np.asarray(head_bias, dtype=np.float32)

    if mode == "fp32":
        feat_in = feats
    else:
        import ml_dtypes
        feat_in = feats.astype(ml_dtypes.bfloat16)

    if mode == "bf16u":
        import ml_dtypes
        in_maps, unperm = [], []   # unperm[k] = device column per position
        for c in range(NCORES):
            allr = mask_idx[c * KH:(c + 1) * KH].ravel()
            idx_payload = np.zeros((NBUCK, P, BS // 16), np.int16)
            ulist = []
            for bb in range(NBUCK):
                u = np.unique(allr[(allr >> shift) == bb])
                assert u.size <= BS, f"union bucket overflow: {u.size}"
                padded = np.zeros(BS, np.int16)
                padded[:u.size] = (u & mask).astype(np.int16)
                idx_payload[bb] = _wrap_idx(padded, BS)
                ulist.append(u)
            for hidx in range(KH):
                gid = mask_idx[c * KH + hidx]
                cols = np.empty(S, np.int64)
                bb_all = gid >> shift
                for bb in range(NBUCK):
                    m = bb_all == bb
                    cols[m] = bb * BS + np.searchsorted(ulist[bb], gid[m])
                unperm.append(cols)
            Wc = W[c * KH:(c + 1) * KH]
            w_in = np.ascontiguousarray(
                Wc.reshape(KH, 2, P, V).transpose(2, 1, 0, 3).reshape(P, 2 * KH * V))
            biasw_in = np.zeros((P, KH * V), np.float32)
            biasw_in[0] = hbias[c * KH:(c + 1) * KH].ravel()
            in_maps.append({
                "feat": feat_in, "idx": idx_payload,
                "w": w_in.astype(ml_dtypes.bfloat16),
                "biasw": biasw_in.astype(ml_dtypes.bfloat16),
                "ones": np.ones((P, 512), ml_dtypes.bfloat16),
            })
        return in_maps, unperm

    in_maps = []
    unperm = []   # per head: (order, counts)
    for c in range(NCORES):
        idx_payload = np.zeros((HB, P, BS // 16), np.int16)
        for hidx, k in enumerate(range(c * KH, (c + 1) * KH)):
            gid = mask_idx[k]
            bidx = gid >> shift
            # sort by full index (not just bucket id): within-bucket ascending
            # order gives the SDMA engines HBM-page-local read streams
            order = np.argsort(gid, kind="stable")
            counts = np.bincount(bidx, minlength=NBUCK)
            assert counts.max() <= BS, f"bucket overflow: {counts}"
            rel = (gid & mask).astype(np.int16)
            pos = 0
            for bb in range(NBUCK):
                cnt = int(counts[bb])
                padded = np.zeros(BS, np.int16)
                padded[:cnt] = rel[order[pos:pos + cnt]]
                idx_payload[hidx * NBUCK + bb] = _wrap_idx(padded, BS)
                pos += cnt
            unperm.append((order, counts))

        Wc = W[c * KH:(c + 1) * KH]          # [KH, 256, 8]
        w_in = np.ascontiguousarray(
            Wc.reshape(KH, 2, P, V).transpose(2, 0, 1, 3).reshape(P, KH * 2 * V))
        if mode == "fp32":
            bias_in = np.ascontiguousarray(hbias[c * KH:(c + 1) * KH].T)  # [V, KH]
            in_maps.append({"feat": feat_in, "idx": idx_payload,
                            "w": w_in, "bias": bias_in})
        else:
            import ml_dtypes
            biasw_in = np.zeros((P, KH * V), np.float32)
            biasw_in[0] = hbias[c * KH:(c + 1) * KH].reshape(KH * V)
            in_maps.append({
                "feat": feat_in, "idx": idx_payload,
                "w": w_in.astype(ml_dtypes.bfloat16),
                "biasw": biasw_in.astype(ml_dtypes.bfloat16),
                "ones": np.ones((P, 512), ml_dtypes.bfloat16),
            })
    return in_maps, unperm


def assemble_output(results, unperm, mode=MODE):
    NBUCK = GEO[mode]["NBUCK"]
    BS = GEO[mode]["BS"]
    out_full = np.empty((NCORES * KH, S, V), np.float32)
    for c in range(NCORES):
        dev = results[c]["out"]              # [KH, V, SW] or [KH*V, SW]
        for h in range(KH):
            k = c * KH + h
            if mode == "bf16u":
                out_full[k] = dev[h * V:(h + 1) * V][:, unperm[k]].T
                continue
            order, counts = unperm[k]
            cols = np.concatenate(
                [np.arange(bb * BS, bb * BS + counts[bb]) for bb in range(NBUCK)])
            out_full[k, order, :] = dev[h][:, cols].T
    return out_full


def kernel(block_type_grid=None, features=None, mask_idx=None,
           head_weights=None, head_bias=None):
    nc = get_nc(MODE)
    in_maps, unperm = prep_inputs(features, mask_idx, head_weights, head_bias, MODE)
    res = run_bass_kernel_spmd(nc, in_maps, list(range(NCORES)))
    return assemble_output(res.results, unperm, MODE)
